# revision 18
# baseline (speedup 1.0000x reference)
"""Expert-parallel MoE routing kernel for 8 Trainium2 NeuronCores.

Reference computation (B=4096, D=1024, O=1024, E=16, K=2):
    router_logits = x @ Wr.T + br                       # [B, E]
    topk_logits, topk_indices = top_k(router_logits, K) # [B, K]
    gate[e, b] = topk logit of expert e for sample b (0 if not selected)
    h = relu(einsum('bd,eod->ebo', x, We) + be)         # [E, B, O]
    out = h * gate[:, :, None]
    returns (out, router_logits, topk_indices)

Sharding: expert-parallel. Each of the 8 cores computes 2 experts' [2, B, O]
output slab plus a replicated copy of the router/top-k (cheap: <2% of FLOPs),
so no collectives are needed. Host-side work is layout-only: x / We / Wr are
pre-transposed so the contraction dim lands on SBUF partitions, and the
per-core output slabs are concatenated along the expert axis at the end.

Device kernel (per core, SPMD over 8 cores):
  - expert matmuls in float32r (1 cyc/row on the PE vs 4 for float32,
    ~1e-4 max rel err), router matmul in full float32 so top-k index
    decisions match the fp32 reference.
  - top-2 + gate construction on DVE via reduce_max / is_ge masks /
    iota-min trick; indices cast to int32 on device.
  - relu+gate fused into one DVE tensor_scalar (max 0, mult gate) per tile,
    PSUM -> SBUF.
Nonzero br/be (zero in this problem's spec) are folded in as extra K=1
matmul rows, traced only when actually nonzero.
"""

import sys

sys.path.insert(0, "/opt/trn_rl_repo")

import numpy as np

import concourse.bacc as bacc
import concourse.mybir as mybir
import concourse.tile as tile
from concourse.bass_utils import run_bass_kernel_spmd

B, D, O, E, TOPK = 4096, 1024, 1024, 16, 2
NCORES = 8
EL = E // NCORES  # experts per core
NB = B // 128  # b-tiles
KT = D // 128  # contraction tiles
NO = O // 512  # moving-dim tiles per expert row block

_cache: dict = {}


def _build(has_br: bool, has_be: bool, reps: int = 1):
    f32 = mybir.dt.float32
    f32r = mybir.dt.float32r
    i32 = mybir.dt.int32
    AO = mybir.AluOpType
    AX = mybir.AxisListType

    nc = bacc.Bacc("TRN2", target_bir_lowering=False, debug=False, num_devices=NCORES)

    xT = nc.dram_tensor("xT", [D, B], f32r, kind="ExternalInput")
    # full-fp32 view of the same data for the router matmul (top-k decisions
    # must match the fp32 reference; fp32r's ~1e-4 error flips near-ties)
    xT32 = nc.dram_tensor("xT32", [D, B], f32, kind="ExternalInput")
    WeT = nc.dram_tensor("WeT", [EL, D, O], f32r, kind="ExternalInput")
    # Wr.T for all 16 experts + this core's 2 experts again as cols 16:18
    WrTc = nc.dram_tensor("WrTc", [D, E + EL], f32, kind="ExternalInput")
    if has_br:
        brv = nc.dram_tensor("brv", [1, E + EL], f32, kind="ExternalInput")
    if has_be:
        bev = nc.dram_tensor("bev", [EL, 1, O], f32r, kind="ExternalInput")

    out = nc.dram_tensor("out", [EL, B, O], f32, kind="ExternalOutput")
    rl = nc.dram_tensor("router_logits", [B, E], f32, kind="ExternalOutput")
    ti = nc.dram_tensor("topk_idx", [B, TOPK], i32, kind="ExternalOutput")

    iota_np = np.broadcast_to(
        np.arange(E, dtype=np.float32) + 65536.0, (128, E)
    ).copy()
    iota_dram = nc.inline_tensor(iota_np, name="iotabig")

    with tile.TileContext(nc) as tc:
        with (
            tc.tile_pool(name="const", bufs=1) as cpool,
            tc.tile_pool(name="wpool", bufs=1) as wpool,
            tc.tile_pool(name="xpool", bufs=3) as xpool,
            tc.tile_pool(name="stage", bufs=4) as spool,
            tc.tile_pool(name="small", bufs=4) as smpool,
            tc.tile_pool(name="psum", bufs=6, space="PSUM") as pp,
            tc.tile_pool(name="rpsum", bufs=2, space="PSUM") as rpp,
        ):
            iot = cpool.tile([128, E], f32, tag="iot")
            nc.sync.dma_start(iot[:], iota_dram[:])
            if has_br:
                onest = cpool.tile([1, 128], f32, tag="ones")
                nc.vector.memset(onest[:], 1.0)
            if has_be:
                onesr = cpool.tile([1, 128], f32r, tag="onesr")
                nc.vector.memset(onesr[:], 1.0)
            wr = cpool.tile([128, KT, E + EL], f32, tag="wr")
            nc.sync.dma_start(wr[:], WrTc[:].rearrange("(k p) e -> p k e", p=128))
            if has_br:
                brt = cpool.tile([1, E + EL], f32, tag="brt")
                nc.sync.dma_start(brt[:], brv[:])
            if has_be:
                bet = cpool.tile([EL, 1, O], f32r, tag="bet")
                nc.sync.dma_start(bet[:], bev[:])

            # resident expert weights: one tile per (e, k), loaded k-major so
            # the first b-tiles can start before the full 8 MB lands
            wt = [[None] * KT for _ in range(EL)]
            for k in range(KT):
                for e in range(EL):
                    t = wpool.tile([128, O], f32r, tag=f"wt{e}_{k}", name=f"wt{e}_{k}")
                    nc.sync.dma_start(
                        t[:], WeT[e, k * 128 : (k + 1) * 128, :].rearrange("p o -> p o")
                    )
                    wt[e][k] = t

            # persistent staging for the small outputs (single DMA at the end)
            rl_stage = cpool.tile([128, NB, E], f32, tag="rl_stage")
            ti_stage = cpool.tile([128, NB, TOPK], i32, tag="ti_stage")

            def body():
                for j in range(NB):
                    _body_j(j)
                nc.sync.dma_start(
                    rl[:].rearrange("(n p) e -> p n e", p=128), rl_stage[:]
                )
                nc.sync.dma_start(
                    ti[:].rearrange("(n p) k -> p n k", p=128), ti_stage[:]
                )

            def _body_j(j):
                xt = xpool.tile([128, KT, 128], f32r, tag="xt", name="xt")
                nc.sync.dma_start(
                    xt[:],
                    xT[:, j * 128 : (j + 1) * 128].rearrange("(k p) b -> p k b", p=128),
                )
                xt32 = xpool.tile([128, KT, 128], f32, tag="xt32", name="xt32")
                nc.sync.dma_start(
                    xt32[:],
                    xT32[:, j * 128 : (j + 1) * 128].rearrange(
                        "(k p) b -> p k b", p=128
                    ),
                )

                rps = rpp.tile([128, E + EL], f32, tag="rps")
                ps = [
                    pp.tile([128, 512], f32, tag="ps", name=f"ps{i}")
                    for i in range(EL * NO)
                ]

                for k in range(KT):
                    nc.tensor.matmul(
                        rps[:],
                        xt32[:, k, :],
                        wr[:, k, :],
                        start=(k == 0),
                        stop=(k == KT - 1) and not has_br,
                    )
                    for e in range(EL):
                        for o in range(NO):
                            nc.tensor.matmul(
                                ps[e * NO + o][:],
                                xt[:, k, :],
                                wt[e][k][:, o * 512 : (o + 1) * 512],
                                start=(k == 0),
                                stop=(k == KT - 1) and not has_be,
                            )
                if has_br:
                    nc.tensor.matmul(
                        rps[:], onest[:], brt[:], start=False, stop=True
                    )
                if has_be:
                    for e in range(EL):
                        for o in range(NO):
                            nc.tensor.matmul(
                                ps[e * NO + o][:],
                                onesr[:],
                                bet[e, :, o * 512 : (o + 1) * 512],
                                start=False,
                                stop=True,
                            )

                # ---- router top-2 + gates (all [128, <=18] ops) ----
                lg = rl_stage[:, j, :]
                nc.scalar.copy(lg, rps[:, 0:E])
                llt = smpool.tile([128, EL], f32, tag="llt")
                nc.scalar.copy(llt[:], rps[:, E : E + EL])

                m1 = smpool.tile([128, 1], f32, tag="m1")
                nc.vector.tensor_reduce(m1[:], lg, axis=AX.X, op=AO.max)
                eq1 = smpool.tile([128, E], f32, tag="eq1")
                nc.vector.tensor_scalar(eq1[:], lg, m1[:], None, AO.is_ge)
                idxc1 = smpool.tile([128, E], f32, tag="idxc1")
                nc.vector.scalar_tensor_tensor(
                    idxc1[:], eq1[:], -65536.0, iot[:], AO.mult, AO.add
                )
                iff = smpool.tile([128, TOPK], f32, tag="iff")
                nc.vector.tensor_reduce(iff[:, 0:1], idxc1[:], axis=AX.X, op=AO.min)

                msk = smpool.tile([128, E], f32, tag="msk")
                nc.vector.scalar_tensor_tensor(
                    msk[:], eq1[:], -1e9, lg, AO.mult, AO.add
                )
                m2 = smpool.tile([128, 1], f32, tag="m2")
                nc.vector.tensor_reduce(m2[:], msk[:], axis=AX.X, op=AO.max)
                eq2 = smpool.tile([128, E], f32, tag="eq2")
                nc.vector.tensor_scalar(eq2[:], msk[:], m2[:], None, AO.is_ge)
                idxc2 = smpool.tile([128, E], f32, tag="idxc2")
                nc.vector.scalar_tensor_tensor(
                    idxc2[:], eq2[:], -65536.0, iot[:], AO.mult, AO.add
                )
                nc.vector.tensor_reduce(iff[:, 1:2], idxc2[:], axis=AX.X, op=AO.min)
                nc.vector.tensor_copy(ti_stage[:, j, :], iff[:])

                # gate for the local experts: logit * (logit >= m2)
                gate = smpool.tile([128, EL], f32, tag="gate")
                nc.vector.scalar_tensor_tensor(
                    gate[:], llt[:], m2[:], llt[:], AO.is_ge, AO.mult
                )

                # ---- relu + gate, PSUM -> SBUF, store ----
                for e in range(EL):
                    ot = spool.tile([128, O], f32, tag="ot")
                    for o in range(NO):
                        nc.vector.tensor_scalar(
                            ot[:, o * 512 : (o + 1) * 512],
                            ps[e * NO + o][:],
                            0.0,
                            gate[:, e : e + 1],
                            AO.max,
                            AO.mult,
                        )
                    nc.sync.dma_start(out[e, j * 128 : (j + 1) * 128, :], ot[:])

            if reps == 1:
                body()
            else:
                with tc.For_i(0, reps, 1):
                    body()

    nc.compile()
    return nc


def run(inputs: dict, trace: bool = False, reps: int = 1):
    x = np.ascontiguousarray(np.asarray(inputs["x"], dtype=np.float32))
    Wr = np.ascontiguousarray(np.asarray(inputs["Wr"], dtype=np.float32))
    br = np.ascontiguousarray(np.asarray(inputs["br"], dtype=np.float32))
    We = np.ascontiguousarray(np.asarray(inputs["We"], dtype=np.float32))
    be = np.ascontiguousarray(np.asarray(inputs["be"], dtype=np.float32))

    has_br = bool(np.any(br))
    has_be = bool(np.any(be))
    key = (has_br, has_be, reps)
    if key not in _cache:
        _cache[key] = _build(has_br, has_be, reps)
    nc = _cache[key]

    xTh = np.ascontiguousarray(x.T)  # [D, B]
    in_maps = []
    for c in range(NCORES):
        lo, hi = c * EL, (c + 1) * EL
        m = {
            "xT": xTh,
            "xT32": xTh,
            "WeT": np.ascontiguousarray(We[lo:hi].transpose(0, 2, 1)),  # [EL, D, O]
            "WrTc": np.ascontiguousarray(
                np.concatenate([Wr.T, Wr[lo:hi].T], axis=1)
            ),  # [D, E+EL]
        }
        if has_br:
            m["brv"] = np.ascontiguousarray(
                np.concatenate([br, br[lo:hi]])[None, :]
            )
        if has_be:
            m["bev"] = np.ascontiguousarray(be[lo:hi][:, None, :])
        in_maps.append(m)

    res = run_bass_kernel_spmd(nc, in_maps, list(range(NCORES)), trace=trace)

    weighted = np.concatenate(
        [res.results[c]["out"] for c in range(NCORES)], axis=0
    )  # [E, B, O]
    router_logits = res.results[0]["router_logits"]  # [B, E] f32
    topk_idx = res.results[0]["topk_idx"].astype(np.int32)  # [B, K]
    return (weighted, router_logits, topk_idx), res


def kernel(**inputs):
    outputs, _ = run(inputs)
    return outputs
